# revision 52
# baseline (speedup 1.0000x reference)
"""Trainium2 Bass kernel for nn_CommunityTimeModel (GNN message passing).

Math: with x = (x_real, x_imag) of shape [N, 1], the [N, H] weighted
scatter-add + complex linear + CSiLU collapses into, per 512-dst tile,
    psum[128 (h,c), 512 d] = sum_blocks lhsT_blk.T @ P_blk
where P = ew * x products (per-edge) and lhsT is a host-built constant
replicating the rank-2 weights over edge-slot rows.  The segment sum and
the linear map both live inside the PE matmul; SiLU on Act; no transpose
anywhere on device.

Sharding: dst-range shard across 8 cores (each core owns 12288 dst
nodes; every edge lands on exactly one core -> no collectives).

Host prep: per core, permute dsts so intra-community-active ones come
first (intra edges are ~1/128 of all), sorted by inter-degree.  Edges are
packed into fp16 "bins" [rows<=128, 1024] = (ew|ew dup rows, xr|xi rows)
x 512 dst columns; each tile's block sits at a 32-aligned row offset
with height 2W (W = max in-degree in tile, chunked at 64).  Bin/tile
structure is the max/union across cores so one SPMD program serves all 8
(missing slots are zero => contribute nothing).

Device per core, pipelined: SP in-DMA per bin -> DVE one fp16 product op
per bin -> PE fp16 matmul per block (accumulating psum per tile) -> Act
SiLU (psum f32 -> fp16) [-> DVE add for intra-active tiles] -> SP
out-DMA.  Output [128 (h,c), 12288 dst] fp16; host transposes,
un-permutes, casts to f32.
"""
from contextlib import ExitStack

import numpy as np

import concourse.bass as bass
import concourse.mybir as mybir
from concourse.bass_utils import run_bass_kernel_spmd

F32 = mybir.dt.float32
F16 = mybir.dt.float16
AF = mybir.ActivationFunctionType
ALU = mybir.AluOpType

N = 98304
NCORES = 8
ND = N // NCORES          # 12288 dst per core
TS = 512                  # dsts per tile
NT = ND // TS             # 24 tiles
WCHUNK = 64               # max W per block (2W <= 128 contraction)
WGRAIN = 4                # W rounding (bounds distinct lhsT count)
NBUF = 8                  # slot/P buffer pool depth
NPSB = 4                  # psumB rotation
NPSL = 2                  # psumL rotation
NOUT = 12                 # outbuf slots (6 out-pairs in flight)


def _round_up(x, m):
    return -(-x // m) * m


class Meta:
    pass


def _plan(degB_all, degI_all, nI_dsts_max):
    """Shared (cross-core max) tile/block/bin structure.

    degB_all, degI_all: [NCORES, ND] degrees in PERMUTED dst order.
    """
    m = Meta()
    m.nI = min(NT, -(-int(nI_dsts_max) // TS)) if nI_dsts_max > 0 else 0

    tmaxB = degB_all.reshape(NCORES, NT, TS).max(axis=(0, 2))
    tmaxI = degI_all.reshape(NCORES, NT, TS).max(axis=(0, 2))

    # blocks in tile order: (tile, kind, chunk, W)
    blocks = []
    for t in range(NT):
        deg = int(tmaxB[t])
        c = 0
        while True:
            w = _round_up(min(max(deg, 1), WCHUNK), WGRAIN)
            blocks.append(dict(t=t, kind="B", c=c, W=w))
            deg -= WCHUNK
            c += 1
            if deg <= 0:
                break
        if t < m.nI:
            deg = int(tmaxI[t])
            c = 0
            while True:
                w = _round_up(min(max(deg, 1), WCHUNK), WGRAIN)
                blocks.append(dict(t=t, kind="I", c=c, W=w))
                deg -= WCHUNK
                c += 1
                if deg <= 0:
                    break

    # Pack blocks into bins of <=128 rows.  Hardware AP rule: base
    # partition must be 0/32/64, and a base-32 AP spans <=32 rows, base-64
    # <=64 rows.  Legal placements: o=0 any H; o=32 H=32; o=64 H<=64.
    # Blocks arrive in tile order; keep one open bin per height class with
    # a span cap so buffer recycling stays tile-local.
    SPAN = 3
    bins = []

    def new_bin(b, h):
        bn = dict(rows=h, blocks=[b], t_first=b["t"])
        b["o"] = 0
        b["_bin"] = bn
        bins.append(bn)
        return bn

    open32 = open64 = None
    for b in blocks:
        h = _round_up(2 * b["W"], 32)
        if h >= 96:
            new_bin(b, h)
        elif h == 64:
            if open64 is not None and b["t"] - open64["t_first"] <= SPAN:
                b["o"] = 64
                b["_bin"] = open64
                open64["rows"] = 128
                open64["blocks"].append(b)
                open64 = None
            else:
                open64 = new_bin(b, h)
        else:
            if (open32 is None or open32["rows"] >= 96
                    or b["t"] - open32["t_first"] > SPAN):
                open32 = new_bin(b, h)
            else:
                b["o"] = open32["rows"]
                b["_bin"] = open32
                open32["rows"] += 32
                open32["blocks"].append(b)

    bins.sort(key=lambda bn: bn["t_first"])
    for j, bn in enumerate(bins):
        bn["idx"] = j
        for b in bn["blocks"]:
            b["bin"] = j
        tB = [b["t"] for b in bn["blocks"] if b["kind"] == "B"]
        tI = [b["t"] for b in bn["blocks"] if b["kind"] == "I"]
        bn["ltB"] = max(tB) if tB else -1
        bn["ltI"] = max(tI) if tI else -1
    m.bins = bins
    m.blocks = blocks

    # per-tile matmul programs
    m.tileB = [[] for _ in range(NT)]
    m.tileI = [[] for _ in range(m.nI)]
    for b in blocks:
        (m.tileB if b["kind"] == "B" else m.tileI)[b["t"]].append(b)
    m.binlastB = [max(b["bin"] for b in m.tileB[t]) for t in range(NT)]
    m.binlastI = [max(b["bin"] for b in m.tileI[t]) for t in range(m.nI)]

    # recycling safety: bin j's buffers wait on bin j-NBUF being consumed;
    # that consumption must not depend on bin j or later
    for j in range(NBUF, len(bins)):
        prev = bins[j - NBUF]
        lt = max(prev["ltB"], prev["ltI"])
        if lt >= bins[j]["t_first"]:
            raise RuntimeError(
                f"bin recycling hazard: bin {j - NBUF} consumed at tile "
                f"{lt} >= bin {j} first use {bins[j]['t_first']}")

    # lhsT const entries: one per (kind, W, row offset) -- matmul requires
    # lhsT and rhs to share the same base partition.  Entries with disjoint
    # row ranges share a 128-row column block.
    m.centry = {}     # (kind, W, o) -> column block index
    cblocks = []      # list of occupied-interval lists
    for b in blocks:
        key = (b["kind"], b["W"], b["o"])
        if key in m.centry:
            continue
        o, h = b["o"], 2 * b["W"]
        placed = False
        for ci, iv in enumerate(cblocks):
            if all(o + h <= s or o >= s + l for s, l in iv):
                iv.append((o, h))
                m.centry[key] = ci
                placed = True
                break
        if not placed:
            cblocks.append([(o, h)])
            m.centry[key] = len(cblocks) - 1
    m.ncon = len(cblocks)
    return m


def _build(m):
    nc = bass.Bass()

    NB = len(m.bins)
    slots = nc.declare_dram_parameter("slots", [128, 1024 * NB], F16,
                                      isOutput=False)
    consts = nc.declare_dram_parameter("consts", [128, 128 * m.ncon], F16,
                                       isOutput=False)
    out = nc.declare_dram_parameter("out", [128, ND], F16, isOutput=True)

    with ExitStack() as ctx:
        e = ctx.enter_context
        slot_sb = [e(nc.sbuf_tensor(f"slot{i}", [128, 1024], F16))
                   for i in range(NBUF)]
        p_sb = [e(nc.sbuf_tensor(f"p{i}", [128, TS], F16))
                for i in range(NBUF)]
        consts_sb = e(nc.sbuf_tensor("consts_sb", [128, 128 * m.ncon], F16))
        outbuf = e(nc.sbuf_tensor("outbuf", [128, NOUT * TS], F16))
        sB_sb = [e(nc.sbuf_tensor(f"sB{i}", [128, TS], F16)) for i in range(2)]
        sL_sb = [e(nc.sbuf_tensor(f"sL{i}", [128, TS], F16)) for i in range(2)]
        psumB = [e(nc.psum_tensor(f"psB{i}", [128, TS], F32))
                 for i in range(NPSB)]
        psumL = [e(nc.psum_tensor(f"psL{i}", [128, TS], F32))
                 for i in range(NPSL)]

        inw = e(nc.semaphore("inw"))
        in_sems = [e(nc.semaphore(f"in{i}")) for i in range(NBUF)]
        p_sem = e(nc.semaphore("p_sem"))
        mmB = e(nc.semaphore("mmB"))
        mmL = e(nc.semaphore("mmL"))
        siluL = e(nc.semaphore("siluL"))
        outrdy = e(nc.semaphore("outrdy"))
        outdones = [e(nc.semaphore(f"outdone{i}")) for i in range(NOUT // 2)]
        block = e(nc.Block())

        # DMA completions are unordered across in-flight DMAs, so each
        # buffer slot gets its own semaphore; the wait value encodes the
        # reuse round.
        def in_wait(eng, j):
            eng.wait_ge(in_sems[j % NBUF], 16 * (j // NBUF + 1))

        # out pair p (tiles 2p, 2p+1) writes outbuf slots 2p%NOUT..+1;
        # freed when its DMA completes
        def outdone_wait(eng, p):
            if p < 0:
                return
            eng.wait_ge(outdones[p % (NOUT // 2)],
                        16 * (p // (NOUT // 2) + 1))

        # bin j's buffers are free once bin j-NBUF's matmuls all ran
        def bin_free_wait(eng, j):
            jj = j - NBUF
            if jj < 0:
                return
            prev = m.bins[jj]
            if prev["ltB"] >= 0:
                eng.wait_ge(mmB, prev["ltB"] + 1)
            if prev["ltI"] >= 0:
                eng.wait_ge(mmL, prev["ltI"] + 1)

        @block.sync
        def _(sync):
            items = [(bn["t_first"], "b", j) for j, bn in enumerate(m.bins)]
            items += [(-1.0, "c", 0)]
            items.sort(key=lambda it: (it[0], it[1]))
            for _, typ, idx in items:
                if typ == "c":
                    sync.dma_start(consts_sb[:], consts[:]).then_inc(inw, 16)
                else:
                    bn = m.bins[idx]
                    bin_free_wait(sync, idx)
                    sync.dma_start(
                        slot_sb[idx % NBUF][0:bn["rows"], :],
                        slots[0:bn["rows"], 1024 * idx:1024 * (idx + 1)],
                    ).then_inc(in_sems[idx % NBUF], 16)

        # out-DMAs live on the (otherwise idle) Pool engine so their
        # readiness waits never stall the bin-DMA stream on SP; pairs keep
        # the tail drain short
        @block.gpsimd
        def _(gpsimd):
            for p in range(NT // 2 - 1):
                gpsimd.wait_ge(outrdy, 2 * p + 2)
                gpsimd.dma_start(
                    out[:, 2 * p * TS:(2 * p + 2) * TS],
                    outbuf[:, (2 * p % NOUT) * TS:((2 * p % NOUT) + 2) * TS],
                ).then_inc(outdones[p % (NOUT // 2)], 16)
            # last pair as two singles: tile NT-2's write overlaps the
            # final silu (nobody waits on the last pair's outdone)
            p = NT // 2 - 1
            for t in (NT - 2, NT - 1):
                gpsimd.wait_ge(outrdy, t + 1)
                gpsimd.dma_start(
                    out[:, t * TS:(t + 1) * TS],
                    outbuf[:, (t % NOUT) * TS:((t % NOUT) + 1) * TS],
                ).then_inc(outdones[p % (NOUT // 2)], 16)

        @block.vector
        def _(vector):
            items = [(bn["t_first"], "b", j) for j, bn in enumerate(m.bins)]
            items += [(t + 3.9, "a", t) for t in range(m.nI)]
            items.sort(key=lambda it: (it[0], it[1]))
            for _, typ, idx in items:
                if typ == "b":
                    bn = m.bins[idx]
                    in_wait(vector, idx)
                    bin_free_wait(vector, idx)
                    sb = slot_sb[idx % NBUF]
                    r = bn["rows"]
                    vector.tensor_tensor(
                        out=p_sb[idx % NBUF][0:r, :], in0=sb[0:r, 0:TS],
                        in1=sb[0:r, TS:2 * TS], op=ALU.mult,
                    ).then_inc(p_sem, 1)
                else:
                    t = idx
                    vector.wait_ge(siluL, t + 1)
                    if t >= 8:
                        outdone_wait(vector, (t - 8) // 2)
                    vector.tensor_tensor(
                        out=outbuf[:, (t % NOUT) * TS:(t % NOUT + 1) * TS],
                        in0=sB_sb[t % 2][:], in1=sL_sb[t % 2][:],
                        op=ALU.add,
                    ).then_inc(outrdy, 1)

        @block.tensor
        def _(tensor):
            tensor.wait_ge(inw, 16)
            for t in range(NT):
                tensor.wait_ge(p_sem, m.binlastB[t] + 1)
                if t >= NPSB:
                    tensor.wait_ge(outrdy, t - NPSB + 1)
                prog = m.tileB[t]
                for k, b in enumerate(prog):
                    w, o = b["W"], b["o"]
                    ci = m.centry[("B", w, o)]
                    ins = tensor.matmul(
                        out=psumB[t % NPSB][:],
                        lhsT=consts_sb[o:o + 2 * w, ci * 128:(ci + 1) * 128],
                        rhs=p_sb[b["bin"] % NBUF][o:o + 2 * w, :],
                        start=(k == 0), stop=(k == len(prog) - 1),
                    )
                ins.then_inc(mmB, 1)
                if t < m.nI:
                    tensor.wait_ge(p_sem, m.binlastI[t] + 1)
                    if t >= NPSL:
                        tensor.wait_ge(siluL, t - NPSL + 1)
                    prog = m.tileI[t]
                    for k, b in enumerate(prog):
                        w, o = b["W"], b["o"]
                        ci = m.centry[("I", w, o)]
                        ins = tensor.matmul(
                            out=psumL[t % NPSL][:],
                            lhsT=consts_sb[o:o + 2 * w,
                                           ci * 128:(ci + 1) * 128],
                            rhs=p_sb[b["bin"] % NBUF][o:o + 2 * w, :],
                            start=(k == 0), stop=(k == len(prog) - 1),
                        )
                    ins.then_inc(mmL, 1)

        @block.scalar
        def _(scalar):
            # preload the Silu activation table during pipeline fill
            scalar.memzero(sB_sb[0][0:1, 0:8])
            scalar.activation(out=sB_sb[0][0:1, 0:8], in_=sB_sb[0][0:1, 0:8],
                              func=AF.Silu)
            for t in range(NT):
                scalar.wait_ge(mmB, t + 1)
                if t < m.nI:
                    if t >= 2:
                        scalar.wait_ge(outrdy, t - 1)
                    scalar.activation(
                        out=sB_sb[t % 2][:], in_=psumB[t % NPSB][:],
                        func=AF.Silu)
                    scalar.wait_ge(mmL, t + 1)
                    scalar.activation(
                        out=sL_sb[t % 2][:], in_=psumL[t % NPSL][:],
                        func=AF.Silu,
                    ).then_inc(siluL, 1)
                else:
                    if t == m.nI and m.nI > 0:
                        scalar.wait_ge(outrdy, m.nI)
                    if t >= 8:
                        outdone_wait(scalar, (t - 8) // 2)
                    scalar.activation(
                        out=outbuf[:, (t % NOUT) * TS:(t % NOUT + 1) * TS],
                        in_=psumB[t % NPSB][:], func=AF.Silu,
                    ).then_inc(outrdy, 1)

    return nc


def _prep(inputs):
    ei = np.asarray(inputs["edge_index"])
    src = np.ascontiguousarray(ei[0]).astype(np.int64)
    dst = np.ascontiguousarray(ei[1]).astype(np.int64)
    ew = np.asarray(inputs["edge_weight"], np.float32)
    comm = np.asarray(inputs["comm_id"], np.int64)
    same = comm[src] == comm[dst]
    xr = np.asarray(inputs["x_real"], np.float32)[:, 0]
    xi = np.asarray(inputs["x_imag"], np.float32)[:, 0]

    Wl_r, Wl_i, Wg_r, Wg_i = (np.asarray(inputs[n], np.float32)[:, 0]
                              for n in ("W_local_r", "W_local_i",
                                        "W_global_r", "W_global_i"))

    core = dst // ND
    d_loc = dst - core * ND

    perms = []
    degB_all = np.zeros((NCORES, ND), np.int64)
    degI_all = np.zeros((NCORES, ND), np.int64)
    nI_max = 0
    core_edges = []
    for k in range(NCORES):
        sel = core == k
        dl = d_loc[sel]
        sm = same[sel]
        degB = np.bincount(dl[~sm], minlength=ND)
        degI = np.bincount(dl[sm], minlength=ND)
        has_i = degI > 0
        order = np.lexsort((-degB, ~has_i))
        inv = np.empty(ND, np.int64)
        inv[order] = np.arange(ND)
        perms.append(order)
        degB_all[k] = degB[order]
        degI_all[k] = degI[order]
        nI_max = max(nI_max, int(has_i.sum()))
        core_edges.append((np.flatnonzero(sel), inv))

    m = _plan(degB_all, degI_all, nI_max)
    NB = len(m.bins)

    # lhsT const table: entry rows [o:o+W] r-coeffs, [o+W:o+2W] i-coeffs
    consts = np.zeros((128, 128 * m.ncon), np.float16)
    for (kind, w, o), ci in m.centry.items():
        wr, wi = (Wg_r, Wg_i) if kind == "B" else (Wl_r, Wl_i)
        blk = np.zeros((2 * w, 128), np.float32)
        blk[0:w, 0::2] = wr
        blk[0:w, 1::2] = wi
        blk[w:2 * w, 0::2] = -wi
        blk[w:2 * w, 1::2] = wr
        consts[o:o + 2 * w, ci * 128:(ci + 1) * 128] = blk.astype(np.float16)

    # lookup: (kind, tile, chunk) -> block
    bkey = {}
    for b in m.blocks:
        bkey[(b["kind"], b["t"], b["c"])] = b
    binarr = np.zeros((2, NT, 8), np.int64)   # bin per (kind, tile, chunk)
    oarr = np.zeros((2, NT, 8), np.int64)
    warr = np.zeros((2, NT, 8), np.int64)
    for b in m.blocks:
        ki = 0 if b["kind"] == "B" else 1
        binarr[ki, b["t"], b["c"]] = b["bin"]
        oarr[ki, b["t"], b["c"]] = b["o"]
        warr[ki, b["t"], b["c"]] = b["W"]

    in_maps = []
    for k in range(NCORES):
        eidx, inv = core_edges[k]
        nd = inv[d_loc[eidx]]
        sm = same[eidx]
        slot = np.zeros((128, 1024 * NB), np.float16)
        scols = 1024 * NB

        for ki, mask in ((0, ~sm), (1, sm)):
            ee = eidx[mask]
            if ee.size == 0:
                continue
            ndm = nd[mask]
            o = np.argsort(ndm, kind="stable")
            ee, ndm = ee[o], ndm[o]
            starts = np.zeros(ndm.size, np.int64)
            first = np.ones(ndm.size, bool)
            first[1:] = ndm[1:] != ndm[:-1]
            starts[first] = np.arange(ndm.size)[first]
            np.maximum.accumulate(starts, out=starts)
            rank = np.arange(ndm.size) - starts
            tile = ndm // TS
            tcol = ndm % TS
            chunk = rank // WCHUNK
            rk = rank - chunk * WCHUNK
            bsel = binarr[ki, tile, chunk]
            osel = oarr[ki, tile, chunk]
            wsel = warr[ki, tile, chunk]
            if (rk >= wsel).any():
                raise RuntimeError("slot overflow")
            col = 1024 * bsel + tcol
            sflat = slot.reshape(-1)
            ewv = ew[ee].astype(np.float16)
            sflat[(osel + rk) * scols + col] = ewv
            sflat[(osel + wsel + rk) * scols + col] = ewv
            sflat[(osel + rk) * scols + col + TS] = xr[src[ee]].astype(np.float16)
            sflat[(osel + wsel + rk) * scols + col + TS] = xi[src[ee]].astype(np.float16)

        in_maps.append({"slots": slot, "consts": consts})
    return in_maps, m, perms


def kernel(**inputs) -> np.ndarray:
    in_maps, m, perms = _prep(inputs)
    nc = _build(m)
    res = run_bass_kernel_spmd(nc, in_maps, list(range(NCORES)))
    full = np.empty((N, 128), np.float32)
    for k in range(NCORES):
        o = np.asarray(res.results[k]["out"]).astype(np.float32).T  # [ND,128]
        full[k * ND + perms[k]] = o
    return full.reshape(N, 64, 2)
